# revision 25
# baseline (speedup 1.0000x reference)
"""BAG-LSTM fused kernel for Trainium2 (Bass/Tile), data-parallel over 8 cores.

v3 strategy (baseline f32r/PE-transpose kernel ran ~917us):
- All GEMM operands bf16, cast host-side (RNE) like the baseline pre-rounded
  to f32r. Halves weight DMA and enables the DMA XBAR transpose path.
- Zero PE transposes: X^T/H0^T tiles come from DRAM via XBAR transpose DMA
  (bf16-only, verified bit-exact on HW). c^T for the BAG GEMMs is re-read the
  same way from the bf16 c spill as 16 full-width transposes into resident
  SBUF tiles (per-m-tile small transposes serialized badly in v2).
- LSTM o-gate mask folded into the spill; ||c||^2 accumulated at LSTM time on
  ACT (Square + accum_out). BAG keeps DVE ops all-fp32 (bf16-sourced operands
  measured 2-3x slower on DVE in v2) by converting main/o to fp32 on ACT.
- LayerNorm gamma/beta: the reference always produces ln_g=1, ln_b=0 (fill
  spec "ones"/"zeros"), so the default NEFF folds them away; a general NEFF
  with the two extra multiplies is compiled lazily if non-identity values
  ever show up.
- Emission order tuned so the first LSTM matmul only waits on x/h0 transposes
  and the first weight slab; BAG weights load after lstm_a is queued.

Numerics: bf16 GEMMs give ~1.1e-2 rel absmax on a_h/v_h (tolerance 2e-2).
"""
import sys

import numpy as np

try:
    import concourse.bacc as bacc
except ImportError:  # fresh-dir grading: repo comes from the container env
    sys.path.insert(0, "/opt/trn_rl_repo")
    import concourse.bacc as bacc

import ml_dtypes
import concourse.mybir as mybir
import concourse.tile as tile
from concourse.bass_utils import run_bass_kernel_spmd
from contextlib import ExitStack

F32 = mybir.dt.float32
BF16 = mybir.dt.bfloat16
Act = mybir.ActivationFunctionType
Alu = mybir.AluOpType

NCORES = 8
B, H = 8192, 1024
BL = B // NCORES          # 1024 batch rows per core
MT = BL // 128            # 8 m-tiles
KT1 = H // 128            # 8  k-tiles for H contraction
KT2 = 2 * H // 128        # 16 k-tiles for 2H contraction
LN_EPS = 1e-5
BAG_EPS = 1e-6


def build(ln_identity=True):
    nc = bacc.Bacc("TRN2", target_bir_lowering=False, debug=False)

    def din(name, shape, dt=F32):
        return nc.dram_tensor(name, shape, dt, kind="ExternalInput")

    def dout(name, shape):
        return nc.dram_tensor(name, shape, F32, kind="ExternalOutput")

    xT_bf = {k: din(f"{k}_xT_bf", [H, BL], BF16) for k in ("a", "v")}
    h0T_bf = {k: din(f"{k}_h0T_bf", [H, BL], BF16) for k in ("a", "v")}
    c0 = {k: din(f"{k}_c0", [BL, H]) for k in ("a", "v")}
    W_sl = {k: din(f"{k}_W_sl", [2, 4, 128, KT2, 512], BF16) for k in ("a", "v")}
    b_f = {k: din(f"{k}_b", [4 * H]) for k in ("a", "v")}
    aco = din("aco_is_rnn_list", [BL, 1])
    vis = din("vis_is_rnn_list", [BL, 1])
    isb = din("is_bag_list", [BL, 1])
    W_mb = din("W_mb_p", [128, KT2, H], BF16)
    b_mb = din("b_mb", [H])
    W_b = din("W_b_p", [128, KT1, H], BF16)
    b_b = din("b_b", [H])
    if not ln_identity:
        ln_g, ln_b = din("ln_g", [H]), din("ln_b", [H])

    a_h, a_sc = dout("a_h", [BL, H]), dout("a_sc", [BL, H])
    v_h, v_sc = dout("v_h", [BL, H]), dout("v_sc", [BL, H])

    # DRAM scratch (per core): bf16 spills of c and ready-masked o
    c_bf = {k: nc.dram_tensor(f"c_{k}_bf", [BL, H], BF16) for k in ("a", "v")}
    o_bf = {k: nc.dram_tensor(f"o_{k}_bf", [BL, H], BF16) for k in ("a", "v")}

    _rr = [0]

    def rr_dma(out, in_):
        engs = (nc.sync, nc.scalar, nc.gpsimd)
        e = engs[_rr[0] % 3]
        _rr[0] += 1
        e.dma_start(out=out, in_=in_)

    def rr2_dma(out, in_):
        # bag-phase round-robin that avoids the sync queue (ct transposes live
        # there; keep the xbar mode stable on it)
        engs = (nc.scalar, nc.gpsimd)
        e = engs[_rr[0] % 2]
        _rr[0] += 1
        e.dma_start(out=out, in_=in_)

    with tile.TileContext(nc) as tc, ExitStack() as ctx:
        consts = ctx.enter_context(tc.tile_pool(name="consts", bufs=1))
        stats = ctx.enter_context(tc.tile_pool(name="stats", bufs=4))
        bagw = ctx.enter_context(tc.tile_pool(name="bagw", bufs=1))

        # per-partition masks [128, MT]: column m = batch rows m*128..m*128+127
        def load_mask(dram):
            t = consts.tile([128, MT], F32, tag=f"mask_{dram.name}")
            nc.gpsimd.dma_start(out=t[:], in_=dram[:].rearrange("(m p) o -> p (m o)", p=128))
            return t

        aco_m = load_mask(aco)
        vis_m = load_mask(vis)
        isb_m = load_mask(isb)
        aco_om = consts.tile([128, MT], F32, tag="aco_om")
        vis_om = consts.tile([128, MT], F32, tag="vis_om")
        nc.vector.tensor_scalar(out=aco_om[:], in0=aco_m[:], scalar1=-1.0,
                                scalar2=1.0, op0=Alu.mult, op1=Alu.add)
        nc.vector.tensor_scalar(out=vis_om[:], in0=vis_m[:], scalar1=-1.0,
                                scalar2=1.0, op0=Alu.mult, op1=Alu.add)
        isb_om = consts.tile([128, MT], F32, tag="isb_om")
        nc.vector.tensor_scalar(out=isb_om[:], in0=isb_m[:], scalar1=-1.0,
                                scalar2=1.0, op0=Alu.mult, op1=Alu.add)

        epsb = consts.tile([128, 1], F32, tag="epsb")
        nc.vector.memset(epsb[:], BAG_EPS)
        epsq = consts.tile([128, 1], F32, tag="epsq")
        nc.vector.memset(epsq[:], BAG_EPS * BAG_EPS)
        epsl = consts.tile([128, 1], F32, tag="epsl")
        nc.vector.memset(epsl[:], LN_EPS)

        if not ln_identity:
            lgb = consts.tile([128, H], F32, tag="lgb")
            nc.gpsimd.dma_start(out=lgb[:], in_=ln_g[:].unsqueeze(0).partition_broadcast(128).squeeze(1))
            lbb = consts.tile([128, H], F32, tag="lbb")
            nc.gpsimd.dma_start(out=lbb[:], in_=ln_b[:].unsqueeze(0).partition_broadcast(128).squeeze(1))

        # ||c||^2 per side, [128, MT], filled during LSTM, read in BAG
        ems = {k: consts.tile([128, MT], F32, tag=f"ems_{k}", name=f"ems_{k}")
               for k in ("a", "v")}
        emn = {k: consts.tile([128, MT], F32, tag=f"emn_{k}", name=f"emn_{k}")
               for k in ("a", "v")}

        # ---------------- LSTM (a then v, shared pools) ----------------
        with ExitStack() as ph:
            xtp = ph.enter_context(tc.tile_pool(name="xt", bufs=2))
            wlp = ph.enter_context(tc.tile_pool(name="wl", bufs=2))
            pap = ph.enter_context(tc.tile_pool(name="pa", bufs=2))
            c0p = ph.enter_context(tc.tile_pool(name="c0", bufs=2))
            gep = ph.enter_context(tc.tile_pool(name="ge", bufs=2))
            bp = ph.enter_context(tc.tile_pool(name="bp", bufs=2))
            spl = ph.enter_context(tc.tile_pool(name="spl", bufs=4))
            gps = ph.enter_context(tc.tile_pool(name="gp", bufs=7, space="PSUM"))

            # X^T / H0^T tiles for BOTH sides up front (XBAR transpose DMA) so
            # the v-side transposes prefetch during lstm_a's gate GEMMs.
            xt = {}
            for tag in ("a", "v"):
                t = xtp.tile([128, KT2, BL], BF16, tag="xt", name=f"xt_{tag}")
                for src, kofs in ((xT_bf[tag], 0), (h0T_bf[tag], KT1)):
                    sv = src[:].rearrange("(k p) m -> p k m", p=128)
                    for q in range(4):
                        k0, k1 = q * 2, q * 2 + 2
                        rr_dma(t[:, kofs + k0:kofs + k1, :], sv[:, k0:k1, :])
                xt[tag] = t

            def lstm_phase(tag, m_col, om_col):
                for ns in range(2):
                    pacc = pap.tile([128, MT, 512], F32, tag="pacc")
                    for gate in (0, 2, 1, 3):      # i, g, f, o
                        cols = gate * H + ns * 512
                        slab = wlp.tile([128, KT2, 512], BF16, tag="wslab")
                        for q in range(4):
                            rr_dma(slab[:, q * 4:(q + 1) * 4, :],
                                   W_sl[tag][ns, gate, :, q * 4:(q + 1) * 4, :])
                        bt = bp.tile([128, 512], F32, tag="brow")
                        rr_dma(bt[:], b_f[tag][cols:cols + 512].unsqueeze(0)
                               .partition_broadcast(128).squeeze(1))
                        for m in range(MT):
                            pt = gps.tile([128, 512], F32, tag="gpt")
                            for k in range(KT2):
                                nc.tensor.matmul(pt[:],
                                                 xt[tag][:, k, m * 128:(m + 1) * 128],
                                                 slab[:, k, :],
                                                 start=(k == 0),
                                                 stop=(k == KT2 - 1))
                            gb = gep.tile([128, 512], F32, tag="gb")
                            nc.vector.tensor_add(gb[:], pt[:], bt[:])
                            if gate == 0:          # i -> P
                                nc.scalar.activation(out=pacc[:, m, :], in_=gb[:],
                                                     func=Act.Sigmoid)
                            elif gate == 2:        # g: P *= tanh(g)
                                nc.scalar.activation(out=gb[:], in_=gb[:],
                                                     func=Act.Tanh)
                                nc.vector.tensor_mul(pacc[:, m, :],
                                                     pacc[:, m, :], gb[:])
                            elif gate == 1:        # f: finish c, spill bf16
                                nc.scalar.activation(out=gb[:], in_=gb[:],
                                                     func=Act.Sigmoid)
                                nc.vector.tensor_scalar(
                                    out=gb[:], in0=gb[:],
                                    scalar1=m_col[:, m:m + 1],
                                    scalar2=om_col[:, m:m + 1],
                                    op0=Alu.mult, op1=Alu.add)
                                c0b = c0p.tile([128, 512], F32, tag="c0b")
                                rr_dma(c0b[:], c0[tag][m * 128:(m + 1) * 128,
                                                       ns * 512:(ns + 1) * 512])
                                nc.vector.tensor_mul(gb[:], gb[:], c0b[:])
                                cbh = spl.tile([128, 512], BF16, tag="cbh", bufs=3)
                                nc.vector.scalar_tensor_tensor(
                                    out=cbh[:], in0=pacc[:, m, :],
                                    scalar=m_col[:, m:m + 1], in1=gb[:],
                                    op0=Alu.mult, op1=Alu.add)
                                # ||c||^2 partial on ACT (junk main output)
                                junk = spl.tile([128, 512], BF16, tag="junk", bufs=2)
                                part = stats.tile([128, 1], F32, tag="emsp")
                                nc.scalar.activation(out=junk[:], in_=cbh[:],
                                                     func=Act.Square,
                                                     accum_out=part[:])
                                if ns == 0:
                                    nc.vector.tensor_copy(
                                        out=ems[tag][:, m:m + 1], in_=part[:])
                                else:
                                    nc.vector.tensor_add(
                                        ems[tag][:, m:m + 1],
                                        ems[tag][:, m:m + 1], part[:])
                                rr_dma(c_bf[tag][m * 128:(m + 1) * 128,
                                                       ns * 512:(ns + 1) * 512], cbh[:])
                            else:                  # o: spill masked sigmoid bf16
                                nc.scalar.activation(out=gb[:], in_=gb[:],
                                                     func=Act.Sigmoid)
                                obh = spl.tile([128, 512], BF16, tag="obh", bufs=2)
                                nc.vector.tensor_scalar(
                                    out=obh[:], in0=gb[:],
                                    scalar1=m_col[:, m:m + 1],
                                    scalar2=om_col[:, m:m + 1],
                                    op0=Alu.mult, op1=Alu.add)
                                rr_dma(o_bf[tag][m * 128:(m + 1) * 128,
                                                       ns * 512:(ns + 1) * 512], obh[:])

            with nc.named_scope("lstm_a"):
                lstm_phase("a", aco_m, aco_om)
            nc.scalar.activation(out=emn["a"][:], in_=ems["a"][:], func=Act.Sqrt)

            # BAG weights load AFTER lstm_a is queued (they're needed ~600us in;
            # loading them first starved the x/h0 transposes in v2)
            wmb = bagw.tile([128, KT2, H], BF16, tag="wmb")
            nc.scalar.dma_start(out=wmb[:], in_=W_mb[:])
            wb_t = bagw.tile([128, KT1, H], BF16, tag="wbt")
            nc.scalar.dma_start(out=wb_t[:], in_=W_b[:])
            bmbb = bagw.tile([128, H], F32, tag="bmbb")
            nc.scalar.dma_start(out=bmbb[:], in_=b_mb[:].unsqueeze(0)
                                .partition_broadcast(128).squeeze(1))
            bbtb = bagw.tile([128, H], F32, tag="bbtb")
            nc.scalar.dma_start(out=bbtb[:], in_=b_b[:].unsqueeze(0)
                                .partition_broadcast(128).squeeze(1))

            with nc.named_scope("lstm_v"):
                lstm_phase("v", vis_m, vis_om)
            nc.scalar.activation(out=emn["v"][:], in_=ems["v"][:], func=Act.Sqrt)

        # ---------------- BAG phase ----------------
        with ExitStack() as ph:
            ctp = ph.enter_context(tc.tile_pool(name="bagct", bufs=2))
            cmp_ = ph.enter_context(tc.tile_pool(name="bagcm", bufs=2))
            orp = ph.enter_context(tc.tile_pool(name="bagor", bufs=2))
            wbp = ph.enter_context(tc.tile_pool(name="bagwb", bufs=2))
            hmp = ph.enter_context(tc.tile_pool(name="baghm", bufs=2))
            jkp = ph.enter_context(tc.tile_pool(name="bagjk", bufs=2))
            bps = ph.enter_context(tc.tile_pool(name="bagps", bufs=8, space="PSUM"))

            # resident c^T for both sides: 16 full-width XBAR transposes
            # (k tiles with ns0 columns depend only on the ns0 spills, so these
            # stream in during lstm_v's second half)
            ct = {}
            for tag in ("a", "v"):
                t = ctp.tile([128, KT1, BL], BF16, tag="ct", name=f"ct_{tag}")
                for k in range(KT1):
                    nc.sync.dma_start(out=t[:, k, :],
                                      in_=c_bf[tag][:, k * 128:(k + 1) * 128],
                                      transpose=True)
                ct[tag] = t

            with nc.named_scope("bag"):
                for m in range(MT):
                    # prefetch masked-o and convert to fp32 ahead of the tail
                    ot = {}
                    for tag, osrc in (("a", o_bf["a"]), ("v", o_bf["v"])):
                        otb = orp.tile([128, H], BF16, tag="otb", name=f"otb_{tag}")
                        rr2_dma(otb[:], osrc[m * 128:(m + 1) * 128, :])
                        otf = orp.tile([128, H], F32, tag="otf", name=f"otf_{tag}")
                        nc.scalar.copy(out=otf[:], in_=otb[:])
                        ot[tag] = otf
                    # main operands in fp32 (bf16-sourced DVE ops are slow)
                    cab = cmp_.tile([128, H], BF16, tag="cab")
                    rr2_dma(cab[:], c_bf["a"][m * 128:(m + 1) * 128, :])
                    ca = cmp_.tile([128, H], F32, tag="ca")
                    nc.scalar.copy(out=ca[:], in_=cab[:])
                    ca_om = cmp_.tile([128, H], F32, tag="ca_om")
                    nc.vector.tensor_scalar_mul(ca_om[:], ca[:], isb_om[:, m:m + 1])
                    cvb = cmp_.tile([128, H], BF16, tag="cvb")
                    rr2_dma(cvb[:], c_bf["v"][m * 128:(m + 1) * 128, :])
                    cv = cmp_.tile([128, H], F32, tag="cv")
                    nc.scalar.copy(out=cv[:], in_=cvb[:])
                    cv_om = cmp_.tile([128, H], F32, tag="cv_om")
                    nc.vector.tensor_scalar_mul(cv_om[:], cv[:], isb_om[:, m:m + 1])

                    def mb_gemm(first, second):
                        ps = []
                        for ns in range(2):
                            p = bps.tile([128, 512], F32, tag="bps")
                            for k in range(KT2):
                                st = (ct[first][:, k, m * 128:(m + 1) * 128]
                                      if k < KT1 else
                                      ct[second][:, k - KT1, m * 128:(m + 1) * 128])
                                nc.tensor.matmul(p[:], st, wmb[:, k, ns * 512:(ns + 1) * 512],
                                                 start=(k == 0), stop=(k == KT2 - 1))
                            ps.append(p)
                        return ps

                    def b_gemm(first):
                        ps = []
                        for ns in range(2):
                            p = bps.tile([128, 512], F32, tag="bps")
                            for k in range(KT1):
                                nc.tensor.matmul(p[:], ct[first][:, k, m * 128:(m + 1) * 128],
                                                 wb_t[:, k, ns * 512:(ns + 1) * 512],
                                                 start=(k == 0), stop=(k == KT1 - 1))
                            ps.append(p)
                        return ps

                    w1 = b_gemm("v")
                    w2 = b_gemm("a")
                    u1 = mb_gemm("a", "v")
                    u2 = mb_gemm("v", "a")

                    def bag_half(u, w, main, main_om, emn_t, out_sc):
                        # weight_b = relu(u); h_m = weight_b * w
                        wbt_ = wbp.tile([128, H], F32, tag="wbrelu")
                        nc.vector.tensor_add(wbt_[:, 0:512], u[0][:], bmbb[:, 0:512])
                        nc.vector.tensor_add(wbt_[:, 512:], u[1][:], bmbb[:, 512:])
                        nc.scalar.activation(out=wbt_[:, 0:512], in_=wbt_[:, 0:512], func=Act.Relu)
                        nc.scalar.activation(out=wbt_[:, 512:], in_=wbt_[:, 512:], func=Act.Relu)
                        wsb = wbp.tile([128, H], F32, tag="wsb")
                        nc.vector.tensor_add(wsb[:, 0:512], w[0][:], bbtb[:, 0:512])
                        nc.vector.tensor_add(wsb[:, 512:], w[1][:], bbtb[:, 512:])
                        hm = hmp.tile([128, H], F32, tag="hm")
                        nc.vector.tensor_mul(hm[:], wbt_[:], wsb[:])
                        # ||h_m||^2 on ACT
                        junk = jkp.tile([128, H], BF16, tag="junkb")
                        hms = stats.tile([128, 1], F32, tag="hms")
                        nc.scalar.activation(out=junk[:], in_=hm[:], func=Act.Square,
                                             accum_out=hms[:])
                        hmn = stats.tile([128, 1], F32, tag="hmn")
                        nc.scalar.activation(out=hmn[:], in_=hms[:], func=Act.Sqrt,
                                             bias=epsq[:], scale=1.0)
                        # alpha = min(emn / sqrt(hms + eps^2), 1)
                        hre = stats.tile([128, 1], F32, tag="hre")
                        nc.vector.reciprocal(out=hre[:], in_=hmn[:])
                        alpha = stats.tile([128, 1], F32, tag="alpha")
                        nc.vector.tensor_scalar(
                            out=alpha[:], in0=emn_t[:, m:m + 1], scalar1=hre[:],
                            scalar2=1.0, op0=Alu.mult, op1=Alu.min)
                        # pre = alpha*hm + main; mean/var via BN stats
                        nc.vector.scalar_tensor_tensor(
                            out=hm[:], in0=hm[:], scalar=alpha[:], in1=main[:],
                            op0=Alu.mult, op1=Alu.add)
                        bn6 = stats.tile([128, 2, 6], F32, tag="bn6")
                        nc.vector.bn_stats(out=bn6[:, 0:1, :], in_=hm[:, 0:512])
                        nc.vector.bn_stats(out=bn6[:, 1:2, :], in_=hm[:, 512:])
                        mv = stats.tile([128, 2], F32, tag="mv")
                        nc.vector.bn_aggr(out=mv[:], in_=bn6[:])
                        nmu = stats.tile([128, 1], F32, tag="nmu")
                        nc.vector.tensor_scalar_mul(nmu[:], mv[:, 0:1], -1.0)
                        rstd = stats.tile([128, 1], F32, tag="rstd")
                        nc.scalar.activation(out=rstd[:], in_=mv[:, 1:2], func=Act.Sqrt,
                                             bias=epsl[:], scale=1.0)
                        nc.vector.reciprocal(out=rstd[:], in_=rstd[:])
                        if not ln_identity:
                            # general LN path: emb = (pre+nmu)*rstd*g + b, then blend
                            nc.vector.tensor_scalar(
                                out=hm[:], in0=hm[:], scalar1=nmu[:], scalar2=rstd[:],
                                op0=Alu.add, op1=Alu.mult)
                            nc.vector.tensor_mul(hm[:], hm[:], lgb[:])
                            nc.vector.tensor_add(hm[:], hm[:], lbb[:])
                            nc.vector.tensor_sub(hm[:], hm[:], main[:])
                            sh = hmp.tile([128, H], F32, tag="sh")
                            nc.vector.scalar_tensor_tensor(
                                out=sh[:], in0=hm[:], scalar=isb_m[:, m:m + 1],
                                in1=main[:], op0=Alu.mult, op1=Alu.add)
                        else:
                            # shift = is_bag*(pre+nmu)*rstd + (1-is_bag)*main
                            t1 = jkp.tile([128, H], F32, tag="t1")
                            nc.vector.tensor_scalar(
                                out=t1[:], in0=hm[:], scalar1=nmu[:], scalar2=rstd[:],
                                op0=Alu.add, op1=Alu.mult)
                            sh = hmp.tile([128, H], F32, tag="sh")
                            nc.vector.scalar_tensor_tensor(
                                out=sh[:], in0=t1[:], scalar=isb_m[:, m:m + 1],
                                in1=main_om[:], op0=Alu.mult, op1=Alu.add)
                        rr2_dma(out_sc[m * 128:(m + 1) * 128, :], sh[:])
                        return sh

                    shifts = [
                        bag_half(u1, w1, ca, ca_om, emn["a"], a_sc),
                        bag_half(u2, w2, cv, cv_om, emn["v"], v_sc)]
                    # h = o_masked * tanh(shift)  (mask folded at LSTM time)
                    for sh, (tag, out_h) in zip(shifts, (("a", a_h), ("v", v_h))):
                        th = jkp.tile([128, H], F32, tag="th")
                        nc.scalar.activation(out=th[:], in_=sh[:], func=Act.Tanh)
                        hh = jkp.tile([128, H], F32, tag="hh")
                        nc.vector.tensor_mul(hh[:], th[:], ot[tag][:])
                        rr2_dma(out_h[m * 128:(m + 1) * 128, :], hh[:])

    nc.compile()
    return nc


_NC = {}


def _get_nc(ln_identity=True):
    if ln_identity not in _NC:
        _NC[ln_identity] = build(ln_identity)
    return _NC[ln_identity]


BATCH_INPUTS = ("aco_is_rnn_list", "vis_is_rnn_list", "is_bag_list",
                "a_c0", "v_c0")
BATCH_T_BF = {"a_xT_bf": "a_x", "a_h0T_bf": "a_h0", "v_xT_bf": "v_x",
              "v_h0T_bf": "v_h0"}
FULL_BF = {}
FULL_F32 = ("a_b", "v_b", "b_mb", "b_b")


def _ln_is_identity(inputs):
    return (np.all(np.asarray(inputs["ln_g"]) == 1.0)
            and np.all(np.asarray(inputs["ln_b"]) == 0.0))


def _pack_slabs(W):
    # [2H, 4H] -> [ns=2, gate=4, p=128, k=KT2, c=512], bf16
    r = W.astype(ml_dtypes.bfloat16).reshape(KT2, 128, 4, 2, 512)
    return np.ascontiguousarray(r.transpose(3, 2, 1, 0, 4))


def _pack_kp(W, kt):
    # [kt*128, N] -> [p=128, k=kt, N], bf16
    r = W.astype(ml_dtypes.bfloat16).reshape(kt, 128, W.shape[1])
    return np.ascontiguousarray(r.transpose(1, 0, 2))


def make_in_maps(inputs, ln_identity=True):
    f32 = {k: np.ascontiguousarray(np.asarray(v), dtype=np.float32)
           for k, v in inputs.items()}
    bf = {n: f32[src].astype(ml_dtypes.bfloat16) for n, src in
          FULL_BF.items()}
    bfT = {n: np.ascontiguousarray(f32[src].astype(ml_dtypes.bfloat16).T)
           for n, src in BATCH_T_BF.items()}
    bf["a_W_sl"] = _pack_slabs(f32["a_W"])
    bf["v_W_sl"] = _pack_slabs(f32["v_W"])
    bf["W_mb_p"] = _pack_kp(f32["W_mb"], KT2)
    bf["W_b_p"] = _pack_kp(f32["W_b"], KT1)
    in_maps = []
    for c in range(NCORES):
        im = {}
        for k in BATCH_INPUTS:
            im[k] = f32[k][c * BL:(c + 1) * BL]
        for k in FULL_F32:
            im[k] = f32[k]
        if not ln_identity:
            im["ln_g"] = f32["ln_g"]
            im["ln_b"] = f32["ln_b"]
        for k in BATCH_T_BF:
            im[k] = np.ascontiguousarray(bfT[k][:, c * BL:(c + 1) * BL])
        for k in FULL_BF:
            im[k] = bf[k]
        for k in ("a_W_sl", "v_W_sl", "W_mb_p", "W_b_p"):
            im[k] = bf[k]
        in_maps.append(im)
    return in_maps


def kernel(**inputs):
    ident = _ln_is_identity(inputs)
    nc = _get_nc(ident)
    in_maps = make_in_maps(inputs, ident)
    res = run_bass_kernel_spmd(nc, in_maps, list(range(NCORES)))
    outs = res.results
    cat = lambda name: np.concatenate([outs[c][name] for c in range(NCORES)], axis=0)
    return (cat("a_h"), cat("a_sc"), cat("v_h"), cat("v_sc"))


# revision 26
# speedup vs baseline: 1.0022x; 1.0022x over previous
"""BAG-LSTM fused kernel for Trainium2 (Bass/Tile), data-parallel over 8 cores.

v3 strategy (baseline f32r/PE-transpose kernel ran ~917us):
- All GEMM operands bf16, cast host-side (RNE) like the baseline pre-rounded
  to f32r. Halves weight DMA and enables the DMA XBAR transpose path.
- Zero PE transposes: X^T/H0^T tiles come from DRAM via XBAR transpose DMA
  (bf16-only, verified bit-exact on HW). c^T for the BAG GEMMs is re-read the
  same way from the bf16 c spill as 16 full-width transposes into resident
  SBUF tiles (per-m-tile small transposes serialized badly in v2).
- LSTM o-gate mask folded into the spill; ||c||^2 accumulated at LSTM time on
  ACT (Square + accum_out). BAG keeps DVE ops all-fp32 (bf16-sourced operands
  measured 2-3x slower on DVE in v2) by converting main/o to fp32 on ACT.
- LayerNorm gamma/beta: the reference always produces ln_g=1, ln_b=0 (fill
  spec "ones"/"zeros"), so the default NEFF folds them away; a general NEFF
  with the two extra multiplies is compiled lazily if non-identity values
  ever show up.
- Emission order tuned so the first LSTM matmul only waits on x/h0 transposes
  and the first weight slab; BAG weights load after lstm_a is queued.

Numerics: bf16 GEMMs give ~1.1e-2 rel absmax on a_h/v_h (tolerance 2e-2).
"""
import sys

import numpy as np

try:
    import concourse.bacc as bacc
except ImportError:  # fresh-dir grading: repo comes from the container env
    sys.path.insert(0, "/opt/trn_rl_repo")
    import concourse.bacc as bacc

import ml_dtypes
import concourse.mybir as mybir
import concourse.tile as tile
from concourse.bass_utils import run_bass_kernel_spmd
from contextlib import ExitStack

F32 = mybir.dt.float32
BF16 = mybir.dt.bfloat16
Act = mybir.ActivationFunctionType
Alu = mybir.AluOpType

NCORES = 8
B, H = 8192, 1024
BL = B // NCORES          # 1024 batch rows per core
MT = BL // 128            # 8 m-tiles
KT1 = H // 128            # 8  k-tiles for H contraction
KT2 = 2 * H // 128        # 16 k-tiles for 2H contraction
LN_EPS = 1e-5
BAG_EPS = 1e-6


def build(ln_identity=True):
    nc = bacc.Bacc("TRN2", target_bir_lowering=False, debug=False)

    def din(name, shape, dt=F32):
        return nc.dram_tensor(name, shape, dt, kind="ExternalInput")

    def dout(name, shape):
        return nc.dram_tensor(name, shape, F32, kind="ExternalOutput")

    xT_bf = {k: din(f"{k}_xT_bf", [H, BL], BF16) for k in ("a", "v")}
    h0T_bf = {k: din(f"{k}_h0T_bf", [H, BL], BF16) for k in ("a", "v")}
    c0 = {k: din(f"{k}_c0", [BL, H]) for k in ("a", "v")}
    W_sl = {k: din(f"{k}_W_sl", [2, 4, 128, KT2, 512], BF16) for k in ("a", "v")}
    b_f = {k: din(f"{k}_b", [4 * H]) for k in ("a", "v")}
    aco = din("aco_is_rnn_list", [BL, 1])
    vis = din("vis_is_rnn_list", [BL, 1])
    isb = din("is_bag_list", [BL, 1])
    W_mb = din("W_mb_p", [128, KT2, H], BF16)
    b_mb = din("b_mb", [H])
    W_b = din("W_b_p", [128, KT1, H], BF16)
    b_b = din("b_b", [H])
    if not ln_identity:
        ln_g, ln_b = din("ln_g", [H]), din("ln_b", [H])

    a_h, a_sc = dout("a_h", [BL, H]), dout("a_sc", [BL, H])
    v_h, v_sc = dout("v_h", [BL, H]), dout("v_sc", [BL, H])

    # DRAM scratch (per core): bf16 spills of c and ready-masked o
    c_bf = {k: nc.dram_tensor(f"c_{k}_bf", [BL, H], BF16) for k in ("a", "v")}
    o_bf = {k: nc.dram_tensor(f"o_{k}_bf", [BL, H], BF16) for k in ("a", "v")}

    _rr = [0]

    def rr_dma(out, in_):
        engs = (nc.sync, nc.scalar, nc.gpsimd)
        e = engs[_rr[0] % 3]
        _rr[0] += 1
        e.dma_start(out=out, in_=in_)

    def rr2_dma(out, in_):
        # bag-phase round-robin that avoids the sync queue (ct transposes live
        # there; keep the xbar mode stable on it)
        engs = (nc.scalar, nc.gpsimd)
        e = engs[_rr[0] % 2]
        _rr[0] += 1
        e.dma_start(out=out, in_=in_)

    with tile.TileContext(nc) as tc, ExitStack() as ctx:
        consts = ctx.enter_context(tc.tile_pool(name="consts", bufs=1))
        stats = ctx.enter_context(tc.tile_pool(name="stats", bufs=4))
        bagw = ctx.enter_context(tc.tile_pool(name="bagw", bufs=1))

        # per-partition masks [128, MT]: column m = batch rows m*128..m*128+127
        def load_mask(dram):
            t = consts.tile([128, MT], F32, tag=f"mask_{dram.name}")
            nc.gpsimd.dma_start(out=t[:], in_=dram[:].rearrange("(m p) o -> p (m o)", p=128))
            return t

        aco_m = load_mask(aco)
        vis_m = load_mask(vis)
        isb_m = load_mask(isb)
        aco_om = consts.tile([128, MT], F32, tag="aco_om")
        vis_om = consts.tile([128, MT], F32, tag="vis_om")
        nc.vector.tensor_scalar(out=aco_om[:], in0=aco_m[:], scalar1=-1.0,
                                scalar2=1.0, op0=Alu.mult, op1=Alu.add)
        nc.vector.tensor_scalar(out=vis_om[:], in0=vis_m[:], scalar1=-1.0,
                                scalar2=1.0, op0=Alu.mult, op1=Alu.add)
        isb_om = consts.tile([128, MT], F32, tag="isb_om")
        nc.vector.tensor_scalar(out=isb_om[:], in0=isb_m[:], scalar1=-1.0,
                                scalar2=1.0, op0=Alu.mult, op1=Alu.add)

        epsb = consts.tile([128, 1], F32, tag="epsb")
        nc.vector.memset(epsb[:], BAG_EPS)
        epsq = consts.tile([128, 1], F32, tag="epsq")
        nc.vector.memset(epsq[:], BAG_EPS * BAG_EPS)
        epsl = consts.tile([128, 1], F32, tag="epsl")
        nc.vector.memset(epsl[:], LN_EPS)

        if not ln_identity:
            lgb = consts.tile([128, H], F32, tag="lgb")
            nc.gpsimd.dma_start(out=lgb[:], in_=ln_g[:].unsqueeze(0).partition_broadcast(128).squeeze(1))
            lbb = consts.tile([128, H], F32, tag="lbb")
            nc.gpsimd.dma_start(out=lbb[:], in_=ln_b[:].unsqueeze(0).partition_broadcast(128).squeeze(1))

        # ||c||^2 per side, [128, MT], filled during LSTM, read in BAG
        ems = {k: consts.tile([128, MT], F32, tag=f"ems_{k}", name=f"ems_{k}")
               for k in ("a", "v")}
        emn = {k: consts.tile([128, MT], F32, tag=f"emn_{k}", name=f"emn_{k}")
               for k in ("a", "v")}

        # ---------------- LSTM (a then v, shared pools) ----------------
        with ExitStack() as ph:
            xtp = ph.enter_context(tc.tile_pool(name="xt", bufs=2))
            wlp = ph.enter_context(tc.tile_pool(name="wl", bufs=2))
            pap = ph.enter_context(tc.tile_pool(name="pa", bufs=2))
            c0p = ph.enter_context(tc.tile_pool(name="c0", bufs=2))
            gep = ph.enter_context(tc.tile_pool(name="ge", bufs=2))
            bp = ph.enter_context(tc.tile_pool(name="bp", bufs=2))
            spl = ph.enter_context(tc.tile_pool(name="spl", bufs=4))
            gps = ph.enter_context(tc.tile_pool(name="gp", bufs=7, space="PSUM"))

            # X^T / H0^T tiles for BOTH sides up front (XBAR transpose DMA) so
            # the v-side transposes prefetch during lstm_a's gate GEMMs.
            xt = {}
            for tag in ("a", "v"):
                t = xtp.tile([128, KT2, BL], BF16, tag="xt", name=f"xt_{tag}")
                for src, kofs in ((xT_bf[tag], 0), (h0T_bf[tag], KT1)):
                    sv = src[:].rearrange("(k p) m -> p k m", p=128)
                    for q in range(4):
                        k0, k1 = q * 2, q * 2 + 2
                        eng = (nc.sync, nc.scalar)[q % 2] if tag == "a" else nc.gpsimd
                        eng.dma_start(out=t[:, kofs + k0:kofs + k1, :],
                                      in_=sv[:, k0:k1, :])
                xt[tag] = t

            def lstm_phase(tag, m_col, om_col):
                for ns in range(2):
                    pacc = pap.tile([128, MT, 512], F32, tag="pacc")
                    for gate in (0, 2, 1, 3):      # i, g, f, o
                        cols = gate * H + ns * 512
                        slab = wlp.tile([128, KT2, 512], BF16, tag="wslab")
                        for q in range(4):
                            eng = (nc.scalar, nc.sync)[q % 2]
                            eng.dma_start(
                                out=slab[:, q * 4:(q + 1) * 4, :],
                                in_=W_sl[tag][ns, gate, :, q * 4:(q + 1) * 4, :])
                        bt = bp.tile([128, 512], F32, tag="brow")
                        nc.scalar.dma_start(
                            out=bt[:],
                            in_=b_f[tag][cols:cols + 512].unsqueeze(0)
                            .partition_broadcast(128).squeeze(1))
                        for m in range(MT):
                            pt = gps.tile([128, 512], F32, tag="gpt")
                            for k in range(KT2):
                                nc.tensor.matmul(pt[:],
                                                 xt[tag][:, k, m * 128:(m + 1) * 128],
                                                 slab[:, k, :],
                                                 start=(k == 0),
                                                 stop=(k == KT2 - 1))
                            gb = gep.tile([128, 512], F32, tag="gb")
                            nc.vector.tensor_add(gb[:], pt[:], bt[:])
                            if gate == 0:          # i -> P
                                nc.scalar.activation(out=pacc[:, m, :], in_=gb[:],
                                                     func=Act.Sigmoid)
                            elif gate == 2:        # g: P *= tanh(g)
                                nc.scalar.activation(out=gb[:], in_=gb[:],
                                                     func=Act.Tanh)
                                nc.vector.tensor_mul(pacc[:, m, :],
                                                     pacc[:, m, :], gb[:])
                            elif gate == 1:        # f: finish c, spill bf16
                                nc.scalar.activation(out=gb[:], in_=gb[:],
                                                     func=Act.Sigmoid)
                                nc.vector.tensor_scalar(
                                    out=gb[:], in0=gb[:],
                                    scalar1=m_col[:, m:m + 1],
                                    scalar2=om_col[:, m:m + 1],
                                    op0=Alu.mult, op1=Alu.add)
                                c0b = c0p.tile([128, 512], F32, tag="c0b")
                                nc.gpsimd.dma_start(
                                    out=c0b[:],
                                    in_=c0[tag][m * 128:(m + 1) * 128,
                                                ns * 512:(ns + 1) * 512])
                                nc.vector.tensor_mul(gb[:], gb[:], c0b[:])
                                cbh = spl.tile([128, 512], BF16, tag="cbh", bufs=3)
                                nc.vector.scalar_tensor_tensor(
                                    out=cbh[:], in0=pacc[:, m, :],
                                    scalar=m_col[:, m:m + 1], in1=gb[:],
                                    op0=Alu.mult, op1=Alu.add)
                                # ||c||^2 partial on ACT (junk main output)
                                junk = spl.tile([128, 512], BF16, tag="junk", bufs=2)
                                part = stats.tile([128, 1], F32, tag="emsp")
                                nc.scalar.activation(out=junk[:], in_=cbh[:],
                                                     func=Act.Square,
                                                     accum_out=part[:])
                                if ns == 0:
                                    nc.vector.tensor_copy(
                                        out=ems[tag][:, m:m + 1], in_=part[:])
                                else:
                                    nc.vector.tensor_add(
                                        ems[tag][:, m:m + 1],
                                        ems[tag][:, m:m + 1], part[:])
                                nc.sync.dma_start(
                                    out=c_bf[tag][m * 128:(m + 1) * 128,
                                                  ns * 512:(ns + 1) * 512],
                                    in_=cbh[:])
                            else:                  # o: spill masked sigmoid bf16
                                nc.scalar.activation(out=gb[:], in_=gb[:],
                                                     func=Act.Sigmoid)
                                obh = spl.tile([128, 512], BF16, tag="obh", bufs=2)
                                nc.vector.tensor_scalar(
                                    out=obh[:], in0=gb[:],
                                    scalar1=m_col[:, m:m + 1],
                                    scalar2=om_col[:, m:m + 1],
                                    op0=Alu.mult, op1=Alu.add)
                                nc.sync.dma_start(
                                    out=o_bf[tag][m * 128:(m + 1) * 128,
                                                  ns * 512:(ns + 1) * 512],
                                    in_=obh[:])

            with nc.named_scope("lstm_a"):
                lstm_phase("a", aco_m, aco_om)
            nc.scalar.activation(out=emn["a"][:], in_=ems["a"][:], func=Act.Sqrt)

            # BAG weights load AFTER lstm_a is queued (they're needed ~600us in;
            # loading them first starved the x/h0 transposes in v2)
            wmb = bagw.tile([128, KT2, H], BF16, tag="wmb")
            nc.scalar.dma_start(out=wmb[:], in_=W_mb[:])
            wb_t = bagw.tile([128, KT1, H], BF16, tag="wbt")
            nc.scalar.dma_start(out=wb_t[:], in_=W_b[:])
            bmbb = bagw.tile([128, H], F32, tag="bmbb")
            nc.scalar.dma_start(out=bmbb[:], in_=b_mb[:].unsqueeze(0)
                                .partition_broadcast(128).squeeze(1))
            bbtb = bagw.tile([128, H], F32, tag="bbtb")
            nc.scalar.dma_start(out=bbtb[:], in_=b_b[:].unsqueeze(0)
                                .partition_broadcast(128).squeeze(1))

            with nc.named_scope("lstm_v"):
                lstm_phase("v", vis_m, vis_om)
            nc.scalar.activation(out=emn["v"][:], in_=ems["v"][:], func=Act.Sqrt)

        # ---------------- BAG phase ----------------
        with ExitStack() as ph:
            ctp = ph.enter_context(tc.tile_pool(name="bagct", bufs=2))
            cmp_ = ph.enter_context(tc.tile_pool(name="bagcm", bufs=2))
            orp = ph.enter_context(tc.tile_pool(name="bagor", bufs=2))
            wbp = ph.enter_context(tc.tile_pool(name="bagwb", bufs=2))
            hmp = ph.enter_context(tc.tile_pool(name="baghm", bufs=2))
            jkp = ph.enter_context(tc.tile_pool(name="bagjk", bufs=2))
            bps = ph.enter_context(tc.tile_pool(name="bagps", bufs=8, space="PSUM"))

            # resident c^T for both sides: 16 full-width XBAR transposes
            # (k tiles with ns0 columns depend only on the ns0 spills, so these
            # stream in during lstm_v's second half)
            ct = {}
            for tag in ("a", "v"):
                t = ctp.tile([128, KT1, BL], BF16, tag="ct", name=f"ct_{tag}")
                for k in range(KT1):
                    nc.sync.dma_start(out=t[:, k, :],
                                      in_=c_bf[tag][:, k * 128:(k + 1) * 128],
                                      transpose=True)
                ct[tag] = t

            with nc.named_scope("bag"):
                for m in range(MT):
                    # prefetch masked-o and convert to fp32 ahead of the tail
                    ot = {}
                    for tag, osrc in (("a", o_bf["a"]), ("v", o_bf["v"])):
                        otb = orp.tile([128, H], BF16, tag="otb", name=f"otb_{tag}")
                        rr2_dma(otb[:], osrc[m * 128:(m + 1) * 128, :])
                        otf = orp.tile([128, H], F32, tag="otf", name=f"otf_{tag}")
                        nc.scalar.copy(out=otf[:], in_=otb[:])
                        ot[tag] = otf
                    # main operands in fp32 (bf16-sourced DVE ops are slow)
                    cab = cmp_.tile([128, H], BF16, tag="cab")
                    rr2_dma(cab[:], c_bf["a"][m * 128:(m + 1) * 128, :])
                    ca = cmp_.tile([128, H], F32, tag="ca")
                    nc.scalar.copy(out=ca[:], in_=cab[:])
                    ca_om = cmp_.tile([128, H], F32, tag="ca_om")
                    nc.vector.tensor_scalar_mul(ca_om[:], ca[:], isb_om[:, m:m + 1])
                    cvb = cmp_.tile([128, H], BF16, tag="cvb")
                    rr2_dma(cvb[:], c_bf["v"][m * 128:(m + 1) * 128, :])
                    cv = cmp_.tile([128, H], F32, tag="cv")
                    nc.scalar.copy(out=cv[:], in_=cvb[:])
                    cv_om = cmp_.tile([128, H], F32, tag="cv_om")
                    nc.vector.tensor_scalar_mul(cv_om[:], cv[:], isb_om[:, m:m + 1])

                    def mb_gemm(first, second):
                        ps = []
                        for ns in range(2):
                            p = bps.tile([128, 512], F32, tag="bps")
                            for k in range(KT2):
                                st = (ct[first][:, k, m * 128:(m + 1) * 128]
                                      if k < KT1 else
                                      ct[second][:, k - KT1, m * 128:(m + 1) * 128])
                                nc.tensor.matmul(p[:], st, wmb[:, k, ns * 512:(ns + 1) * 512],
                                                 start=(k == 0), stop=(k == KT2 - 1))
                            ps.append(p)
                        return ps

                    def b_gemm(first):
                        ps = []
                        for ns in range(2):
                            p = bps.tile([128, 512], F32, tag="bps")
                            for k in range(KT1):
                                nc.tensor.matmul(p[:], ct[first][:, k, m * 128:(m + 1) * 128],
                                                 wb_t[:, k, ns * 512:(ns + 1) * 512],
                                                 start=(k == 0), stop=(k == KT1 - 1))
                            ps.append(p)
                        return ps

                    w1 = b_gemm("v")
                    w2 = b_gemm("a")
                    u1 = mb_gemm("a", "v")
                    u2 = mb_gemm("v", "a")

                    def bag_half(u, w, main, main_om, emn_t, out_sc):
                        # weight_b = relu(u); h_m = weight_b * w
                        wbt_ = wbp.tile([128, H], F32, tag="wbrelu")
                        nc.vector.tensor_add(wbt_[:, 0:512], u[0][:], bmbb[:, 0:512])
                        nc.vector.tensor_add(wbt_[:, 512:], u[1][:], bmbb[:, 512:])
                        nc.scalar.activation(out=wbt_[:, 0:512], in_=wbt_[:, 0:512], func=Act.Relu)
                        nc.scalar.activation(out=wbt_[:, 512:], in_=wbt_[:, 512:], func=Act.Relu)
                        wsb = wbp.tile([128, H], F32, tag="wsb")
                        nc.vector.tensor_add(wsb[:, 0:512], w[0][:], bbtb[:, 0:512])
                        nc.vector.tensor_add(wsb[:, 512:], w[1][:], bbtb[:, 512:])
                        hm = hmp.tile([128, H], F32, tag="hm")
                        nc.vector.tensor_mul(hm[:], wbt_[:], wsb[:])
                        # ||h_m||^2 on ACT
                        junk = jkp.tile([128, H], BF16, tag="junkb")
                        hms = stats.tile([128, 1], F32, tag="hms")
                        nc.scalar.activation(out=junk[:], in_=hm[:], func=Act.Square,
                                             accum_out=hms[:])
                        hmn = stats.tile([128, 1], F32, tag="hmn")
                        nc.scalar.activation(out=hmn[:], in_=hms[:], func=Act.Sqrt,
                                             bias=epsq[:], scale=1.0)
                        # alpha = min(emn / sqrt(hms + eps^2), 1)
                        hre = stats.tile([128, 1], F32, tag="hre")
                        nc.vector.reciprocal(out=hre[:], in_=hmn[:])
                        alpha = stats.tile([128, 1], F32, tag="alpha")
                        nc.vector.tensor_scalar(
                            out=alpha[:], in0=emn_t[:, m:m + 1], scalar1=hre[:],
                            scalar2=1.0, op0=Alu.mult, op1=Alu.min)
                        # pre = alpha*hm + main; mean/var via BN stats
                        nc.vector.scalar_tensor_tensor(
                            out=hm[:], in0=hm[:], scalar=alpha[:], in1=main[:],
                            op0=Alu.mult, op1=Alu.add)
                        bn6 = stats.tile([128, 2, 6], F32, tag="bn6")
                        nc.vector.bn_stats(out=bn6[:, 0:1, :], in_=hm[:, 0:512])
                        nc.vector.bn_stats(out=bn6[:, 1:2, :], in_=hm[:, 512:])
                        mv = stats.tile([128, 2], F32, tag="mv")
                        nc.vector.bn_aggr(out=mv[:], in_=bn6[:])
                        nmu = stats.tile([128, 1], F32, tag="nmu")
                        nc.vector.tensor_scalar_mul(nmu[:], mv[:, 0:1], -1.0)
                        rstd = stats.tile([128, 1], F32, tag="rstd")
                        nc.scalar.activation(out=rstd[:], in_=mv[:, 1:2], func=Act.Sqrt,
                                             bias=epsl[:], scale=1.0)
                        nc.vector.reciprocal(out=rstd[:], in_=rstd[:])
                        if not ln_identity:
                            # general LN path: emb = (pre+nmu)*rstd*g + b, then blend
                            nc.vector.tensor_scalar(
                                out=hm[:], in0=hm[:], scalar1=nmu[:], scalar2=rstd[:],
                                op0=Alu.add, op1=Alu.mult)
                            nc.vector.tensor_mul(hm[:], hm[:], lgb[:])
                            nc.vector.tensor_add(hm[:], hm[:], lbb[:])
                            nc.vector.tensor_sub(hm[:], hm[:], main[:])
                            sh = hmp.tile([128, H], F32, tag="sh")
                            nc.vector.scalar_tensor_tensor(
                                out=sh[:], in0=hm[:], scalar=isb_m[:, m:m + 1],
                                in1=main[:], op0=Alu.mult, op1=Alu.add)
                        else:
                            # shift = is_bag*(pre+nmu)*rstd + (1-is_bag)*main
                            t1 = jkp.tile([128, H], F32, tag="t1")
                            nc.vector.tensor_scalar(
                                out=t1[:], in0=hm[:], scalar1=nmu[:], scalar2=rstd[:],
                                op0=Alu.add, op1=Alu.mult)
                            sh = hmp.tile([128, H], F32, tag="sh")
                            nc.vector.scalar_tensor_tensor(
                                out=sh[:], in0=t1[:], scalar=isb_m[:, m:m + 1],
                                in1=main_om[:], op0=Alu.mult, op1=Alu.add)
                        rr2_dma(out_sc[m * 128:(m + 1) * 128, :], sh[:])
                        return sh

                    shifts = [
                        bag_half(u1, w1, ca, ca_om, emn["a"], a_sc),
                        bag_half(u2, w2, cv, cv_om, emn["v"], v_sc)]
                    # h = o_masked * tanh(shift)  (mask folded at LSTM time)
                    for sh, (tag, out_h) in zip(shifts, (("a", a_h), ("v", v_h))):
                        th = jkp.tile([128, H], F32, tag="th")
                        nc.scalar.activation(out=th[:], in_=sh[:], func=Act.Tanh)
                        hh = jkp.tile([128, H], F32, tag="hh")
                        nc.vector.tensor_mul(hh[:], th[:], ot[tag][:])
                        rr2_dma(out_h[m * 128:(m + 1) * 128, :], hh[:])

    nc.compile()
    return nc


_NC = {}


def _get_nc(ln_identity=True):
    if ln_identity not in _NC:
        _NC[ln_identity] = build(ln_identity)
    return _NC[ln_identity]


BATCH_INPUTS = ("aco_is_rnn_list", "vis_is_rnn_list", "is_bag_list",
                "a_c0", "v_c0")
BATCH_T_BF = {"a_xT_bf": "a_x", "a_h0T_bf": "a_h0", "v_xT_bf": "v_x",
              "v_h0T_bf": "v_h0"}
FULL_BF = {}
FULL_F32 = ("a_b", "v_b", "b_mb", "b_b")


def _ln_is_identity(inputs):
    return (np.all(np.asarray(inputs["ln_g"]) == 1.0)
            and np.all(np.asarray(inputs["ln_b"]) == 0.0))


def _pack_slabs(W):
    # [2H, 4H] -> [ns=2, gate=4, p=128, k=KT2, c=512], bf16
    r = W.astype(ml_dtypes.bfloat16).reshape(KT2, 128, 4, 2, 512)
    return np.ascontiguousarray(r.transpose(3, 2, 1, 0, 4))


def _pack_kp(W, kt):
    # [kt*128, N] -> [p=128, k=kt, N], bf16
    r = W.astype(ml_dtypes.bfloat16).reshape(kt, 128, W.shape[1])
    return np.ascontiguousarray(r.transpose(1, 0, 2))


def make_in_maps(inputs, ln_identity=True):
    f32 = {k: np.ascontiguousarray(np.asarray(v), dtype=np.float32)
           for k, v in inputs.items()}
    bf = {n: f32[src].astype(ml_dtypes.bfloat16) for n, src in
          FULL_BF.items()}
    bfT = {n: np.ascontiguousarray(f32[src].astype(ml_dtypes.bfloat16).T)
           for n, src in BATCH_T_BF.items()}
    bf["a_W_sl"] = _pack_slabs(f32["a_W"])
    bf["v_W_sl"] = _pack_slabs(f32["v_W"])
    bf["W_mb_p"] = _pack_kp(f32["W_mb"], KT2)
    bf["W_b_p"] = _pack_kp(f32["W_b"], KT1)
    in_maps = []
    for c in range(NCORES):
        im = {}
        for k in BATCH_INPUTS:
            im[k] = f32[k][c * BL:(c + 1) * BL]
        for k in FULL_F32:
            im[k] = f32[k]
        if not ln_identity:
            im["ln_g"] = f32["ln_g"]
            im["ln_b"] = f32["ln_b"]
        for k in BATCH_T_BF:
            im[k] = np.ascontiguousarray(bfT[k][:, c * BL:(c + 1) * BL])
        for k in FULL_BF:
            im[k] = bf[k]
        for k in ("a_W_sl", "v_W_sl", "W_mb_p", "W_b_p"):
            im[k] = bf[k]
        in_maps.append(im)
    return in_maps


def kernel(**inputs):
    ident = _ln_is_identity(inputs)
    nc = _get_nc(ident)
    in_maps = make_in_maps(inputs, ident)
    res = run_bass_kernel_spmd(nc, in_maps, list(range(NCORES)))
    outs = res.results
    cat = lambda name: np.concatenate([outs[c][name] for c in range(NCORES)], axis=0)
    return (cat("a_h"), cat("a_sc"), cat("v_h"), cat("v_sc"))


# revision 27
# speedup vs baseline: 1.0484x; 1.0461x over previous
"""BAG-LSTM fused kernel for Trainium2 (Bass/Tile), data-parallel over 8 cores.

v3 strategy (baseline f32r/PE-transpose kernel ran ~917us):
- All GEMM operands bf16, cast host-side (RNE) like the baseline pre-rounded
  to f32r. Halves weight DMA and enables the DMA XBAR transpose path.
- Zero PE transposes: X^T/H0^T tiles come from DRAM via XBAR transpose DMA
  (bf16-only, verified bit-exact on HW). c^T for the BAG GEMMs is re-read the
  same way from the bf16 c spill as 16 full-width transposes into resident
  SBUF tiles (per-m-tile small transposes serialized badly in v2).
- LSTM o-gate mask folded into the spill; ||c||^2 accumulated at LSTM time on
  ACT (Square + accum_out). BAG keeps DVE ops all-fp32 (bf16-sourced operands
  measured 2-3x slower on DVE in v2) by converting main/o to fp32 on ACT.
- LayerNorm gamma/beta: the reference always produces ln_g=1, ln_b=0 (fill
  spec "ones"/"zeros"), so the default NEFF folds them away; a general NEFF
  with the two extra multiplies is compiled lazily if non-identity values
  ever show up.
- Emission order tuned so the first LSTM matmul only waits on x/h0 transposes
  and the first weight slab; BAG weights load after lstm_a is queued.

Numerics: bf16 GEMMs give ~1.1e-2 rel absmax on a_h/v_h (tolerance 2e-2).
"""
import sys

import numpy as np

try:
    import concourse.bacc as bacc
except ImportError:  # fresh-dir grading: repo comes from the container env
    sys.path.insert(0, "/opt/trn_rl_repo")
    import concourse.bacc as bacc

import ml_dtypes
import concourse.mybir as mybir
import concourse.tile as tile
from concourse.bass_utils import run_bass_kernel_spmd
from contextlib import ExitStack

F32 = mybir.dt.float32
BF16 = mybir.dt.bfloat16
Act = mybir.ActivationFunctionType
Alu = mybir.AluOpType

NCORES = 8
B, H = 8192, 1024
BL = B // NCORES          # 1024 batch rows per core
MT = BL // 128            # 8 m-tiles
KT1 = H // 128            # 8  k-tiles for H contraction
KT2 = 2 * H // 128        # 16 k-tiles for 2H contraction
LN_EPS = 1e-5
BAG_EPS = 1e-6


def build(ln_identity=True):
    nc = bacc.Bacc("TRN2", target_bir_lowering=False, debug=False)

    def din(name, shape, dt=F32):
        return nc.dram_tensor(name, shape, dt, kind="ExternalInput")

    def dout(name, shape):
        return nc.dram_tensor(name, shape, F32, kind="ExternalOutput")

    xT_bf = {k: din(f"{k}_xT_bf", [H, BL], BF16) for k in ("a", "v")}
    h0T_bf = {k: din(f"{k}_h0T_bf", [H, BL], BF16) for k in ("a", "v")}
    c0 = {k: din(f"{k}_c0", [BL, H]) for k in ("a", "v")}
    W_sl = {k: din(f"{k}_W_sl", [2, 4, 128, KT2, 512], BF16) for k in ("a", "v")}
    b_f = {k: din(f"{k}_b", [4 * H]) for k in ("a", "v")}
    aco = din("aco_is_rnn_list", [BL, 1])
    vis = din("vis_is_rnn_list", [BL, 1])
    isb = din("is_bag_list", [BL, 1])
    W_mb = din("W_mb_p", [128, KT2, H], BF16)
    b_mb = din("b_mb", [H])
    W_b = din("W_b_p", [128, KT1, H], BF16)
    b_b = din("b_b", [H])
    if not ln_identity:
        ln_g, ln_b = din("ln_g", [H]), din("ln_b", [H])

    a_h, a_sc = dout("a_h", [BL, H]), dout("a_sc", [BL, H])
    v_h, v_sc = dout("v_h", [BL, H]), dout("v_sc", [BL, H])

    # DRAM scratch (per core): bf16 spills of c and ready-masked o
    c_bf = {k: nc.dram_tensor(f"c_{k}_bf", [BL, H], BF16) for k in ("a", "v")}
    o_bf = {k: nc.dram_tensor(f"o_{k}_bf", [BL, H], BF16) for k in ("a", "v")}

    _rr = [0]

    def rr_dma(out, in_):
        engs = (nc.sync, nc.scalar, nc.gpsimd)
        e = engs[_rr[0] % 3]
        _rr[0] += 1
        e.dma_start(out=out, in_=in_)

    def rr2_dma(out, in_):
        # bag-phase round-robin that avoids the sync queue (ct transposes live
        # there; keep the xbar mode stable on it)
        engs = (nc.scalar, nc.gpsimd)
        e = engs[_rr[0] % 2]
        _rr[0] += 1
        e.dma_start(out=out, in_=in_)

    with tile.TileContext(nc) as tc, ExitStack() as ctx:
        consts = ctx.enter_context(tc.tile_pool(name="consts", bufs=1))
        stats = ctx.enter_context(tc.tile_pool(name="stats", bufs=4))
        bagw = ctx.enter_context(tc.tile_pool(name="bagw", bufs=1))

        # per-partition masks [128, MT]: column m = batch rows m*128..m*128+127
        def load_mask(dram):
            t = consts.tile([128, MT], F32, tag=f"mask_{dram.name}")
            nc.gpsimd.dma_start(out=t[:], in_=dram[:].rearrange("(m p) o -> p (m o)", p=128))
            return t

        aco_m = load_mask(aco)
        vis_m = load_mask(vis)
        isb_m = load_mask(isb)
        aco_om = consts.tile([128, MT], F32, tag="aco_om")
        vis_om = consts.tile([128, MT], F32, tag="vis_om")
        nc.vector.tensor_scalar(out=aco_om[:], in0=aco_m[:], scalar1=-1.0,
                                scalar2=1.0, op0=Alu.mult, op1=Alu.add)
        nc.vector.tensor_scalar(out=vis_om[:], in0=vis_m[:], scalar1=-1.0,
                                scalar2=1.0, op0=Alu.mult, op1=Alu.add)
        isb_om = consts.tile([128, MT], F32, tag="isb_om")
        nc.vector.tensor_scalar(out=isb_om[:], in0=isb_m[:], scalar1=-1.0,
                                scalar2=1.0, op0=Alu.mult, op1=Alu.add)

        epsb = consts.tile([128, 1], F32, tag="epsb")
        nc.vector.memset(epsb[:], BAG_EPS)
        epsq = consts.tile([128, 1], F32, tag="epsq")
        nc.vector.memset(epsq[:], BAG_EPS * BAG_EPS)
        epsl = consts.tile([128, 1], F32, tag="epsl")
        nc.vector.memset(epsl[:], LN_EPS)

        if not ln_identity:
            lgb = consts.tile([128, H], F32, tag="lgb")
            nc.gpsimd.dma_start(out=lgb[:], in_=ln_g[:].unsqueeze(0).partition_broadcast(128).squeeze(1))
            lbb = consts.tile([128, H], F32, tag="lbb")
            nc.gpsimd.dma_start(out=lbb[:], in_=ln_b[:].unsqueeze(0).partition_broadcast(128).squeeze(1))

        # ||c||^2 per side, [128, MT], filled during LSTM, read in BAG
        ems = {k: consts.tile([128, MT], F32, tag=f"ems_{k}", name=f"ems_{k}")
               for k in ("a", "v")}
        emn = {k: consts.tile([128, MT], F32, tag=f"emn_{k}", name=f"emn_{k}")
               for k in ("a", "v")}

        # ---------------- LSTM (a then v, shared pools) ----------------
        with ExitStack() as ph:
            xtp = ph.enter_context(tc.tile_pool(name="xt", bufs=2))
            wlp = ph.enter_context(tc.tile_pool(name="wl", bufs=2))
            pap = ph.enter_context(tc.tile_pool(name="pa", bufs=2))
            c0p = ph.enter_context(tc.tile_pool(name="c0", bufs=2))
            gep = ph.enter_context(tc.tile_pool(name="ge", bufs=2))
            bp = ph.enter_context(tc.tile_pool(name="bp", bufs=2))
            spl = ph.enter_context(tc.tile_pool(name="spl", bufs=4))
            gps = ph.enter_context(tc.tile_pool(name="gp", bufs=7, space="PSUM"))

            # X^T / H0^T tiles for BOTH sides up front (XBAR transpose DMA) so
            # the v-side transposes prefetch during lstm_a's gate GEMMs.
            xt = {}
            for tag in ("a", "v"):
                t = xtp.tile([128, KT2, BL], BF16, tag="xt", name=f"xt_{tag}")
                for src, kofs in ((xT_bf[tag], 0), (h0T_bf[tag], KT1)):
                    sv = src[:].rearrange("(k p) m -> p k m", p=128)
                    for q in range(4):
                        k0, k1 = q * 2, q * 2 + 2
                        eng = (nc.sync, nc.scalar)[q % 2] if tag == "a" else nc.gpsimd
                        eng.dma_start(out=t[:, kofs + k0:kofs + k1, :],
                                      in_=sv[:, k0:k1, :])
                xt[tag] = t

            def lstm_phase(tag, m_col, om_col):
                for ns in range(2):
                    pacc = pap.tile([128, MT, 512], F32, tag="pacc")
                    for gate in (0, 2, 1, 3):      # i, g, f, o
                        cols = gate * H + ns * 512
                        slab = wlp.tile([128, KT2, 512], BF16, tag="wslab")
                        for q in range(4):
                            nc.scalar.dma_start(
                                out=slab[:, q * 4:(q + 1) * 4, :],
                                in_=W_sl[tag][ns, gate, :, q * 4:(q + 1) * 4, :])
                        bt = bp.tile([128, 512], F32, tag="brow")
                        nc.scalar.dma_start(
                            out=bt[:],
                            in_=b_f[tag][cols:cols + 512].unsqueeze(0)
                            .partition_broadcast(128).squeeze(1))
                        for m in range(MT):
                            pt = gps.tile([128, 512], F32, tag="gpt")
                            for k in range(KT2):
                                nc.tensor.matmul(pt[:],
                                                 xt[tag][:, k, m * 128:(m + 1) * 128],
                                                 slab[:, k, :],
                                                 start=(k == 0),
                                                 stop=(k == KT2 - 1))
                            gb = gep.tile([128, 512], F32, tag="gb")
                            nc.vector.tensor_add(gb[:], pt[:], bt[:])
                            if gate == 0:          # i -> P
                                nc.scalar.activation(out=pacc[:, m, :], in_=gb[:],
                                                     func=Act.Sigmoid)
                            elif gate == 2:        # g: P *= tanh(g)
                                nc.scalar.activation(out=gb[:], in_=gb[:],
                                                     func=Act.Tanh)
                                nc.vector.tensor_mul(pacc[:, m, :],
                                                     pacc[:, m, :], gb[:])
                            elif gate == 1:        # f: finish c, spill bf16
                                nc.scalar.activation(out=gb[:], in_=gb[:],
                                                     func=Act.Sigmoid)
                                nc.vector.tensor_scalar(
                                    out=gb[:], in0=gb[:],
                                    scalar1=m_col[:, m:m + 1],
                                    scalar2=om_col[:, m:m + 1],
                                    op0=Alu.mult, op1=Alu.add)
                                c0b = c0p.tile([128, 512], F32, tag="c0b")
                                nc.gpsimd.dma_start(
                                    out=c0b[:],
                                    in_=c0[tag][m * 128:(m + 1) * 128,
                                                ns * 512:(ns + 1) * 512])
                                nc.vector.tensor_mul(gb[:], gb[:], c0b[:])
                                cbh = spl.tile([128, 512], BF16, tag="cbh", bufs=3)
                                nc.vector.scalar_tensor_tensor(
                                    out=cbh[:], in0=pacc[:, m, :],
                                    scalar=m_col[:, m:m + 1], in1=gb[:],
                                    op0=Alu.mult, op1=Alu.add)
                                # ||c||^2 partial on ACT (junk main output)
                                junk = spl.tile([128, 512], BF16, tag="junk", bufs=2)
                                part = stats.tile([128, 1], F32, tag="emsp")
                                nc.scalar.activation(out=junk[:], in_=cbh[:],
                                                     func=Act.Square,
                                                     accum_out=part[:])
                                if ns == 0:
                                    nc.vector.tensor_copy(
                                        out=ems[tag][:, m:m + 1], in_=part[:])
                                else:
                                    nc.vector.tensor_add(
                                        ems[tag][:, m:m + 1],
                                        ems[tag][:, m:m + 1], part[:])
                                nc.sync.dma_start(
                                    out=c_bf[tag][m * 128:(m + 1) * 128,
                                                  ns * 512:(ns + 1) * 512],
                                    in_=cbh[:])
                            else:                  # o: spill masked sigmoid bf16
                                nc.scalar.activation(out=gb[:], in_=gb[:],
                                                     func=Act.Sigmoid)
                                obh = spl.tile([128, 512], BF16, tag="obh", bufs=2)
                                nc.vector.tensor_scalar(
                                    out=obh[:], in0=gb[:],
                                    scalar1=m_col[:, m:m + 1],
                                    scalar2=om_col[:, m:m + 1],
                                    op0=Alu.mult, op1=Alu.add)
                                nc.sync.dma_start(
                                    out=o_bf[tag][m * 128:(m + 1) * 128,
                                                  ns * 512:(ns + 1) * 512],
                                    in_=obh[:])

            with nc.named_scope("lstm_a"):
                lstm_phase("a", aco_m, aco_om)
            nc.scalar.activation(out=emn["a"][:], in_=ems["a"][:], func=Act.Sqrt)

            # BAG weights load AFTER lstm_a is queued (they're needed ~600us in;
            # loading them first starved the x/h0 transposes in v2)
            wmb = bagw.tile([128, KT2, H], BF16, tag="wmb")
            nc.scalar.dma_start(out=wmb[:], in_=W_mb[:])
            wb_t = bagw.tile([128, KT1, H], BF16, tag="wbt")
            nc.scalar.dma_start(out=wb_t[:], in_=W_b[:])
            bmbb = bagw.tile([128, H], F32, tag="bmbb")
            nc.scalar.dma_start(out=bmbb[:], in_=b_mb[:].unsqueeze(0)
                                .partition_broadcast(128).squeeze(1))
            bbtb = bagw.tile([128, H], F32, tag="bbtb")
            nc.scalar.dma_start(out=bbtb[:], in_=b_b[:].unsqueeze(0)
                                .partition_broadcast(128).squeeze(1))

            with nc.named_scope("lstm_v"):
                lstm_phase("v", vis_m, vis_om)
            nc.scalar.activation(out=emn["v"][:], in_=ems["v"][:], func=Act.Sqrt)

        # ---------------- BAG phase ----------------
        with ExitStack() as ph:
            ctp = ph.enter_context(tc.tile_pool(name="bagct", bufs=2))
            cmp_ = ph.enter_context(tc.tile_pool(name="bagcm", bufs=2))
            orp = ph.enter_context(tc.tile_pool(name="bagor", bufs=2))
            wbp = ph.enter_context(tc.tile_pool(name="bagwb", bufs=2))
            hmp = ph.enter_context(tc.tile_pool(name="baghm", bufs=2))
            jkp = ph.enter_context(tc.tile_pool(name="bagjk", bufs=2))
            bps = ph.enter_context(tc.tile_pool(name="bagps", bufs=8, space="PSUM"))

            # resident c^T for both sides: 16 full-width XBAR transposes
            # (k tiles with ns0 columns depend only on the ns0 spills, so these
            # stream in during lstm_v's second half)
            ct = {}
            for tag in ("a", "v"):
                t = ctp.tile([128, KT1, BL], BF16, tag="ct", name=f"ct_{tag}")
                for k in range(KT1):
                    nc.sync.dma_start(out=t[:, k, :],
                                      in_=c_bf[tag][:, k * 128:(k + 1) * 128],
                                      transpose=True)
                ct[tag] = t

            with nc.named_scope("bag"):
                for m in range(MT):
                    # prefetch masked-o and convert to fp32 ahead of the tail
                    ot = {}
                    for tag, osrc in (("a", o_bf["a"]), ("v", o_bf["v"])):
                        otb = orp.tile([128, H], BF16, tag="otb", name=f"otb_{tag}")
                        nc.sync.dma_start(out=otb[:],
                                          in_=osrc[m * 128:(m + 1) * 128, :])
                        otf = orp.tile([128, H], F32, tag="otf", name=f"otf_{tag}")
                        nc.scalar.copy(out=otf[:], in_=otb[:])
                        ot[tag] = otf
                    # main operands in fp32 (bf16-sourced DVE ops are slow)
                    cab = cmp_.tile([128, H], BF16, tag="cab")
                    nc.sync.dma_start(out=cab[:], in_=c_bf["a"][m * 128:(m + 1) * 128, :])
                    ca = cmp_.tile([128, H], F32, tag="ca")
                    nc.scalar.copy(out=ca[:], in_=cab[:])
                    ca_om = cmp_.tile([128, H], F32, tag="ca_om")
                    nc.vector.tensor_scalar_mul(ca_om[:], ca[:], isb_om[:, m:m + 1])
                    cvb = cmp_.tile([128, H], BF16, tag="cvb")
                    nc.sync.dma_start(out=cvb[:], in_=c_bf["v"][m * 128:(m + 1) * 128, :])
                    cv = cmp_.tile([128, H], F32, tag="cv")
                    nc.scalar.copy(out=cv[:], in_=cvb[:])
                    cv_om = cmp_.tile([128, H], F32, tag="cv_om")
                    nc.vector.tensor_scalar_mul(cv_om[:], cv[:], isb_om[:, m:m + 1])

                    def mb_gemm(first, second):
                        ps = []
                        for ns in range(2):
                            p = bps.tile([128, 512], F32, tag="bps")
                            for k in range(KT2):
                                st = (ct[first][:, k, m * 128:(m + 1) * 128]
                                      if k < KT1 else
                                      ct[second][:, k - KT1, m * 128:(m + 1) * 128])
                                nc.tensor.matmul(p[:], st, wmb[:, k, ns * 512:(ns + 1) * 512],
                                                 start=(k == 0), stop=(k == KT2 - 1))
                            ps.append(p)
                        return ps

                    def b_gemm(first):
                        ps = []
                        for ns in range(2):
                            p = bps.tile([128, 512], F32, tag="bps")
                            for k in range(KT1):
                                nc.tensor.matmul(p[:], ct[first][:, k, m * 128:(m + 1) * 128],
                                                 wb_t[:, k, ns * 512:(ns + 1) * 512],
                                                 start=(k == 0), stop=(k == KT1 - 1))
                            ps.append(p)
                        return ps

                    w1 = b_gemm("v")
                    w2 = b_gemm("a")
                    u1 = mb_gemm("a", "v")
                    u2 = mb_gemm("v", "a")

                    def bag_half(u, w, main, main_om, emn_t, out_sc):
                        # weight_b = relu(u); h_m = weight_b * w
                        wbt_ = wbp.tile([128, H], F32, tag="wbrelu")
                        nc.vector.tensor_add(wbt_[:, 0:512], u[0][:], bmbb[:, 0:512])
                        nc.vector.tensor_add(wbt_[:, 512:], u[1][:], bmbb[:, 512:])
                        nc.scalar.activation(out=wbt_[:, 0:512], in_=wbt_[:, 0:512], func=Act.Relu)
                        nc.scalar.activation(out=wbt_[:, 512:], in_=wbt_[:, 512:], func=Act.Relu)
                        wsb = wbp.tile([128, H], F32, tag="wsb")
                        nc.vector.tensor_add(wsb[:, 0:512], w[0][:], bbtb[:, 0:512])
                        nc.vector.tensor_add(wsb[:, 512:], w[1][:], bbtb[:, 512:])
                        hm = hmp.tile([128, H], F32, tag="hm")
                        nc.vector.tensor_mul(hm[:], wbt_[:], wsb[:])
                        # ||h_m||^2 on ACT
                        junk = jkp.tile([128, H], BF16, tag="junkb")
                        hms = stats.tile([128, 1], F32, tag="hms")
                        nc.scalar.activation(out=junk[:], in_=hm[:], func=Act.Square,
                                             accum_out=hms[:])
                        hmn = stats.tile([128, 1], F32, tag="hmn")
                        nc.scalar.activation(out=hmn[:], in_=hms[:], func=Act.Sqrt,
                                             bias=epsq[:], scale=1.0)
                        # alpha = min(emn / sqrt(hms + eps^2), 1)
                        hre = stats.tile([128, 1], F32, tag="hre")
                        nc.vector.reciprocal(out=hre[:], in_=hmn[:])
                        alpha = stats.tile([128, 1], F32, tag="alpha")
                        nc.vector.tensor_scalar(
                            out=alpha[:], in0=emn_t[:, m:m + 1], scalar1=hre[:],
                            scalar2=1.0, op0=Alu.mult, op1=Alu.min)
                        # pre = alpha*hm + main; mean/var via BN stats
                        nc.vector.scalar_tensor_tensor(
                            out=hm[:], in0=hm[:], scalar=alpha[:], in1=main[:],
                            op0=Alu.mult, op1=Alu.add)
                        bn6 = stats.tile([128, 2, 6], F32, tag="bn6")
                        nc.vector.bn_stats(out=bn6[:, 0:1, :], in_=hm[:, 0:512])
                        nc.vector.bn_stats(out=bn6[:, 1:2, :], in_=hm[:, 512:])
                        mv = stats.tile([128, 2], F32, tag="mv")
                        nc.vector.bn_aggr(out=mv[:], in_=bn6[:])
                        nmu = stats.tile([128, 1], F32, tag="nmu")
                        nc.vector.tensor_scalar_mul(nmu[:], mv[:, 0:1], -1.0)
                        rstd = stats.tile([128, 1], F32, tag="rstd")
                        nc.scalar.activation(out=rstd[:], in_=mv[:, 1:2], func=Act.Sqrt,
                                             bias=epsl[:], scale=1.0)
                        nc.vector.reciprocal(out=rstd[:], in_=rstd[:])
                        if not ln_identity:
                            # general LN path: emb = (pre+nmu)*rstd*g + b, then blend
                            nc.vector.tensor_scalar(
                                out=hm[:], in0=hm[:], scalar1=nmu[:], scalar2=rstd[:],
                                op0=Alu.add, op1=Alu.mult)
                            nc.vector.tensor_mul(hm[:], hm[:], lgb[:])
                            nc.vector.tensor_add(hm[:], hm[:], lbb[:])
                            nc.vector.tensor_sub(hm[:], hm[:], main[:])
                            sh = hmp.tile([128, H], F32, tag="sh")
                            nc.vector.scalar_tensor_tensor(
                                out=sh[:], in0=hm[:], scalar=isb_m[:, m:m + 1],
                                in1=main[:], op0=Alu.mult, op1=Alu.add)
                        else:
                            # shift = is_bag*(pre+nmu)*rstd + (1-is_bag)*main
                            t1 = jkp.tile([128, H], F32, tag="t1")
                            nc.vector.tensor_scalar(
                                out=t1[:], in0=hm[:], scalar1=nmu[:], scalar2=rstd[:],
                                op0=Alu.add, op1=Alu.mult)
                            sh = hmp.tile([128, H], F32, tag="sh")
                            nc.vector.scalar_tensor_tensor(
                                out=sh[:], in0=t1[:], scalar=isb_m[:, m:m + 1],
                                in1=main_om[:], op0=Alu.mult, op1=Alu.add)
                        nc.sync.dma_start(out=out_sc[m * 128:(m + 1) * 128, :], in_=sh[:])
                        return sh

                    shifts = [
                        bag_half(u1, w1, ca, ca_om, emn["a"], a_sc),
                        bag_half(u2, w2, cv, cv_om, emn["v"], v_sc)]
                    # h = o_masked * tanh(shift)  (mask folded at LSTM time)
                    for sh, (tag, out_h) in zip(shifts, (("a", a_h), ("v", v_h))):
                        th = jkp.tile([128, H], F32, tag="th")
                        nc.scalar.activation(out=th[:], in_=sh[:], func=Act.Tanh)
                        hh = jkp.tile([128, H], F32, tag="hh")
                        nc.vector.tensor_mul(hh[:], th[:], ot[tag][:])
                        nc.sync.dma_start(out=out_h[m * 128:(m + 1) * 128, :], in_=hh[:])

    nc.compile()
    return nc


_NC = {}


def _get_nc(ln_identity=True):
    if ln_identity not in _NC:
        _NC[ln_identity] = build(ln_identity)
    return _NC[ln_identity]


BATCH_INPUTS = ("aco_is_rnn_list", "vis_is_rnn_list", "is_bag_list",
                "a_c0", "v_c0")
BATCH_T_BF = {"a_xT_bf": "a_x", "a_h0T_bf": "a_h0", "v_xT_bf": "v_x",
              "v_h0T_bf": "v_h0"}
FULL_BF = {}
FULL_F32 = ("a_b", "v_b", "b_mb", "b_b")


def _ln_is_identity(inputs):
    return (np.all(np.asarray(inputs["ln_g"]) == 1.0)
            and np.all(np.asarray(inputs["ln_b"]) == 0.0))


def _pack_slabs(W):
    # [2H, 4H] -> [ns=2, gate=4, p=128, k=KT2, c=512], bf16
    r = W.astype(ml_dtypes.bfloat16).reshape(KT2, 128, 4, 2, 512)
    return np.ascontiguousarray(r.transpose(3, 2, 1, 0, 4))


def _pack_kp(W, kt):
    # [kt*128, N] -> [p=128, k=kt, N], bf16
    r = W.astype(ml_dtypes.bfloat16).reshape(kt, 128, W.shape[1])
    return np.ascontiguousarray(r.transpose(1, 0, 2))


def make_in_maps(inputs, ln_identity=True):
    f32 = {k: np.ascontiguousarray(np.asarray(v), dtype=np.float32)
           for k, v in inputs.items()}
    bf = {n: f32[src].astype(ml_dtypes.bfloat16) for n, src in
          FULL_BF.items()}
    bfT = {n: np.ascontiguousarray(f32[src].astype(ml_dtypes.bfloat16).T)
           for n, src in BATCH_T_BF.items()}
    bf["a_W_sl"] = _pack_slabs(f32["a_W"])
    bf["v_W_sl"] = _pack_slabs(f32["v_W"])
    bf["W_mb_p"] = _pack_kp(f32["W_mb"], KT2)
    bf["W_b_p"] = _pack_kp(f32["W_b"], KT1)
    in_maps = []
    for c in range(NCORES):
        im = {}
        for k in BATCH_INPUTS:
            im[k] = f32[k][c * BL:(c + 1) * BL]
        for k in FULL_F32:
            im[k] = f32[k]
        if not ln_identity:
            im["ln_g"] = f32["ln_g"]
            im["ln_b"] = f32["ln_b"]
        for k in BATCH_T_BF:
            im[k] = np.ascontiguousarray(bfT[k][:, c * BL:(c + 1) * BL])
        for k in FULL_BF:
            im[k] = bf[k]
        for k in ("a_W_sl", "v_W_sl", "W_mb_p", "W_b_p"):
            im[k] = bf[k]
        in_maps.append(im)
    return in_maps


def kernel(**inputs):
    ident = _ln_is_identity(inputs)
    nc = _get_nc(ident)
    in_maps = make_in_maps(inputs, ident)
    res = run_bass_kernel_spmd(nc, in_maps, list(range(NCORES)))
    outs = res.results
    cat = lambda name: np.concatenate([outs[c][name] for c in range(NCORES)], axis=0)
    return (cat("a_h"), cat("a_sc"), cat("v_h"), cat("v_sc"))
